# revision 10
# baseline (speedup 1.0000x reference)
"""CurvatureMap Trainium2 kernel.

Computes, per batch image: channel-mean -> 3x3 Sobel-family stencils
(replicate padding) -> Gaussian/mean curvature maps (K, H, kappa).

Sharding: pure data parallel, batch b -> NeuronCore b (8 cores, full H per
core, no halo exchange).

Per-core pipeline (x_b is (64, 512, 512) f32):
  1. mean over 64 channels, one 128-row block at a time, on TensorE:
     64 accumulating float32r matmuls with stationary I/64 weights
     (identity passes the partition index through, PSUM accumulates the
     channel sum) -> mean block lands partition-aligned in PSUM.
  2. separable stencil: vertical 1D convs need row +-1 -> partition-shifted
     SBUF->SBUF DMA halo tiles; horizontal 1D convs are shifted free-axis
     slices. All conv weights are powers of two, folded into downstream
     constants.
  3. pointwise curvature math split across VectorE (fused
     scalar_tensor_tensor ops, Newton-iterated reciprocal) and ScalarE
     (Square/Sqrt/Abs with folded scales).
"""

import numpy as np

import concourse.bacc as bacc
import concourse.bass as bass
import concourse.tile as tile
from concourse import mybir
from concourse.bass_utils import run_bass_kernel_spmd

B, C, H, W = 8, 64, 512, 512
P = 128              # SBUF partitions = rows per block
NB = H // P          # row blocks per core
CHD = 8              # channels per DMA batch
Wp = W + 2           # width incl. replicate padding
F32 = mybir.dt.float32
F32R = mybir.dt.float32r
Alu = mybir.AluOpType
Act = mybir.ActivationFunctionType


def _build_nc():
    nc = bacc.Bacc()
    x_h = nc.dram_tensor("x", [C, H, W], F32R, kind="ExternalInput")
    ab_h = nc.dram_tensor("ab", [P, 2], F32, kind="ExternalInput")
    out_h = nc.dram_tensor("out", [3, H, W], F32, kind="ExternalOutput")
    eye_h = nc.inline_tensor(np.eye(P, dtype=np.float32) / C, name="eye64")

    x = x_h[:, :, :].rearrange("c h w -> h c w")
    lo = slice(0, W)
    ce = slice(1, W + 1)
    hi = slice(2, W + 2)

    with tile.TileContext(nc) as tc:
        with (
            tc.tile_pool(name="per", bufs=1) as per,
            tc.tile_pool(name="rhsp", bufs=3) as rhsp,
            tc.tile_pool(name="wk", bufs=2) as wk,
            tc.tile_pool(name="psp", bufs=4, space="PSUM") as psp,
        ):
            eye_sb = per.tile([P, P], F32R, tag="eye", bufs=1)
            nc.sync.dma_start(out=eye_sb, in_=eye_h[:, :].bitcast(F32R))
            ab_sb = per.tile([P, 2], F32, tag="ab", bufs=1)
            nc.sync.dma_start(out=ab_sb, in_=ab_h[:, :])
            alpha_col = ab_sb[:, 0:1]
            beta_col = ab_sb[:, 1:2]

            # Mean image, all 4 blocks, horizontally edge-padded.
            Mall = per.tile([P, NB, Wp], F32, tag="mall", bufs=1)

            def mean_block(k):
                ps = psp.tile([P, W], F32, tag="ps", bufs=4, name="ps")
                for g in range(C // CHD):
                    rt = rhsp.tile([P, CHD, W], F32R, tag="rhs", bufs=3, name="rt")
                    nc.sync.dma_start(
                        out=rt,
                        in_=x[k * P:(k + 1) * P, g * CHD:(g + 1) * CHD, :],
                    )
                    for ci in range(CHD):
                        ch = g * CHD + ci
                        nc.tensor.matmul(
                            ps,
                            lhsT=eye_sb,
                            rhs=rt[:, ci, :],
                            start=(ch == 0),
                            stop=(ch == C - 1),
                        )
                nc.scalar.copy(out=Mall[:, k, ce], in_=ps)
                nc.scalar.copy(out=Mall[:, k, 0:1], in_=Mall[:, k, 1:2])
                nc.scalar.copy(
                    out=Mall[:, k, W + 1:W + 2], in_=Mall[:, k, W:W + 1]
                )

            def stencil_block(k):
                T1 = Mall[:, k, :]
                # T0[p] = mean row (k*128 + p - 1), T2[p] = row (k*128 + p + 1)
                # (edge rows replicated) -- partition-shifted SBUF->SBUF DMA.
                T0 = wk.tile([P, Wp], F32, tag="T0", bufs=2)
                T2 = wk.tile([P, Wp], F32, tag="T2", bufs=2)
                up = Mall[P - 1:P, k - 1, :] if k > 0 else Mall[0:1, 0, :]
                nc.sync.dma_start(out=T0[0:1, :], in_=up)
                nc.sync.dma_start(out=T0[1:P, :], in_=Mall[0:P - 1, k, :])
                nc.sync.dma_start(out=T2[0:P - 1, :], in_=Mall[1:P, k, :])
                dn = (
                    Mall[0:1, k + 1, :]
                    if k < NB - 1
                    else Mall[P - 1:P, NB - 1, :]
                )
                nc.sync.dma_start(out=T2[P - 1:P, :], in_=dn)

                def wt(tag, w=W):
                    return wk.tile([P, w], F32, tag=tag, bufs=2, name=tag)

                # vertical 1D convs (s=[1,2,1]/4, d=[-1,0,1]/2, d2=[1,-2,1])
                # kept unscaled: Vs4 = 4*vconv_s, Bv = 2*vconv_d, Vd2 = vconv_d2
                A = wt("A", Wp)
                nc.gpsimd.tensor_add(A, T0, T2)
                Bv = wt("Bv", Wp)
                nc.gpsimd.tensor_sub(Bv, T2, T0)
                Vs4 = wt("Vs4", Wp)
                nc.vector.scalar_tensor_tensor(
                    Vs4, in0=T1, scalar=2.0, in1=A, op0=Alu.mult, op1=Alu.add
                )
                Vd2 = wt("Vd2", Wp)
                nc.vector.scalar_tensor_tensor(
                    Vd2, in0=T1, scalar=-2.0, in1=A, op0=Alu.mult, op1=Alu.add
                )

                # horizontal 1D convs, unscaled:
                # sx = 8*I_x, sy = 8*I_y, sxx = 4*I_xx, sxy = 4*I_xy, syy = 4*I_yy
                sx = wt("sx")
                nc.vector.tensor_sub(sx, Vs4[:, hi], Vs4[:, lo])
                SA = wt("SA")
                nc.vector.tensor_add(SA, Vs4[:, lo], Vs4[:, hi])
                sxx = wt("sxx")
                nc.vector.scalar_tensor_tensor(
                    sxx, in0=Vs4[:, ce], scalar=-2.0, in1=SA,
                    op0=Alu.mult, op1=Alu.add,
                )
                BA = wt("BA")
                nc.gpsimd.tensor_add(BA, Bv[:, lo], Bv[:, hi])
                sy = wt("sy")
                nc.vector.scalar_tensor_tensor(
                    sy, in0=Bv[:, ce], scalar=2.0, in1=BA,
                    op0=Alu.mult, op1=Alu.add,
                )
                sxy = wt("sxy")
                nc.vector.tensor_sub(sxy, Bv[:, hi], Bv[:, lo])
                DA = wt("DA")
                nc.gpsimd.tensor_add(DA, Vd2[:, lo], Vd2[:, hi])
                syy = wt("syy")
                nc.vector.scalar_tensor_tensor(
                    syy, in0=Vd2[:, ce], scalar=2.0, in1=DA,
                    op0=Alu.mult, op1=Alu.add,
                )

                # pointwise curvature (scales folded: Ix=sx/8, Ixx=sxx/4, ...)
                x2 = wt("x2")
                nc.scalar.activation(x2, sx, Act.Square, scale=0.125)
                y2 = wt("y2")
                nc.scalar.activation(y2, sy, Act.Square, scale=0.125)
                g_ = wt("g_")
                nc.vector.scalar_tensor_tensor(
                    g_, in0=x2, scalar=1.0, in1=y2, op0=Alu.add, op1=Alu.add
                )
                g2 = wt("g2")
                nc.scalar.activation(g2, g_, Act.Square)
                scr = wt("scr")
                rg2 = wt("rg2")
                nc.vector.reciprocal_approx_accurate(out=rg2, in_=g2, scratch=scr)
                p1 = wt("p1")
                nc.vector.tensor_mul(p1, sxx, syy)
                q = wt("q")
                nc.scalar.activation(q, sxy, Act.Square, scale=0.25)
                Kn = wt("Kn")
                nc.vector.scalar_tensor_tensor(
                    Kn, in0=p1, scalar=0.0625, in1=q,
                    op0=Alu.mult, op1=Alu.subtract,
                )
                K = wt("K")
                nc.vector.tensor_mul(K, Kn, rg2)
                a1 = wt("a1")
                nc.vector.scalar_tensor_tensor(
                    a1, in0=x2, scalar=1.0, in1=syy, op0=Alu.add, op1=Alu.mult
                )
                a2 = wt("a2")
                nc.vector.scalar_tensor_tensor(
                    a2, in0=y2, scalar=1.0, in1=sxx, op0=Alu.add, op1=Alu.mult
                )
                t3 = wt("t3")
                nc.vector.tensor_add(t3, a1, a2)
                u = wt("u")
                nc.vector.tensor_mul(u, sx, sy)
                v = wt("v")
                nc.vector.tensor_mul(v, u, sxy)
                Hn4 = wt("Hn4")
                nc.vector.scalar_tensor_tensor(
                    Hn4, in0=v, scalar=-0.03125, in1=t3,
                    op0=Alu.mult, op1=Alu.add,
                )
                sg = wt("sg")
                nc.scalar.activation(sg, g_, Act.Sqrt)
                m1 = wt("m1")
                nc.vector.tensor_mul(m1, Hn4, rg2)
                Hv = wt("Hv")
                nc.vector.scalar_tensor_tensor(
                    Hv, in0=m1, scalar=0.125, in1=sg, op0=Alu.mult, op1=Alu.mult
                )
                aK = wt("aK")
                nc.scalar.activation(aK, K, Act.Abs)
                aH = wt("aH")
                nc.scalar.activation(aH, Hv, Act.Abs)
                mK = wt("mK")
                nc.vector.tensor_scalar_mul(mK, aK, alpha_col)
                kap = wt("kap")
                nc.vector.scalar_tensor_tensor(
                    kap, in0=aH, scalar=beta_col, in1=mK,
                    op0=Alu.mult, op1=Alu.add,
                )

                rows = slice(k * P, (k + 1) * P)
                nc.sync.dma_start(out=out_h[0, rows, :], in_=K)
                nc.sync.dma_start(out=out_h[1, rows, :], in_=Hv)
                nc.sync.dma_start(out=out_h[2, rows, :], in_=kap)

            # Interleave: stencil(k-1) right after mean(k) so stencil DVE work
            # overlaps the remaining mean DMAs (stencil k needs means k-1..k+1).
            mean_block(0)
            for k in range(1, NB):
                mean_block(k)
                stencil_block(k - 1)
            stencil_block(NB - 1)
    return nc


_CACHE = {}


def _get_nc():
    if "nc" not in _CACHE:
        nc = _build_nc()
        nc.finalize()
        _CACHE["nc"] = nc
    return _CACHE["nc"]


def run(x, alpha, beta, **spmd_kwargs):
    x = np.ascontiguousarray(np.asarray(x, dtype=np.float32))
    assert x.shape == (B, C, H, W), x.shape
    ab = np.empty((P, 2), np.float32)
    ab[:, 0] = np.float32(alpha)
    ab[:, 1] = np.float32(beta)
    nc = _get_nc()
    in_maps = [{"x": x[b], "ab": ab} for b in range(B)]
    res = run_bass_kernel_spmd(nc, in_maps, core_ids=list(range(B)), **spmd_kwargs)
    outs = np.stack([r["out"] for r in res.results])  # (B, 3, H, W)
    K = np.ascontiguousarray(outs[:, 0:1])
    Hm = np.ascontiguousarray(outs[:, 1:2])
    kap = np.ascontiguousarray(outs[:, 2:3])
    return (K, Hm, kap), res


def kernel(x, alpha, beta):
    (K, Hm, kap), _ = run(x, alpha, beta)
    return (K, Hm, kap)
